# revision 14
# baseline (speedup 1.0000x reference)
"""CrossTrackAttention Trainium2 kernel (8-core SPMD, batch x head-group sharding).

Reference computation (B=2, S=2048, D=1024, H=16, HD=64):
    qkv = x @ w_qkv + b_qkv
    q, k, v per head; scores = q k^T / sqrt(HD); masked softmax with a
    [B, S, S] bool mask; out = (attn @ v) @ w_out + b_out.

Sharding: core c handles batch c//4 and heads [4*(c%4), 4*(c%4)+4).  The
[B,H,S,S] score tensor partitions cleanly along B and H, so there are no
cross-device comms; the per-core partial outputs (each over 4 heads' feature
rows of w_out) are summed on the host.

Device algorithm per core (transpose-free flash attention):
  - host passes x^T, so QKV projections produce q^T/k^T in [feature, token]
    layout directly (lhsT = w block, rhs = x^T block) and v in natural
    [token, feature] layout (lhsT = x^T block, rhs = w block).
  - scores are computed transposed, s^T[k, q] (lhsT = k^T slice, rhs = q^T
    slice), so softmax numerator exp(s - 20) runs on ACT over wide-q tiles
    and the attention@V matmul consumes p^T tiles as lhsT with no transposes.
  - a ones column appended to V accumulates the softmax denominator in the
    same PSUM accumulation group; out = p~ @ [v|1] then row-scaled by the
    reciprocal of the denominator (softmax is shift invariant, so the fixed
    -20 shift cancels).
  - 1/sqrt(HD) is folded into w_q/b_q on the host; b_v is folded into the
    output bias (sum_k softmax = 1  =>  +b_v passes through attention), so
    the device never touches b_v.

Two compiled variants:
  - "structured": the cross-track mask of the reference's setup_inputs()
    (causal within each of 2 tracks of 1024 tokens + bidirectional same-bar
    cross-track attention, BAR=64).  Block-sparse schedule with a constant
    128x128 triangular tile for the causal diagonal; no mask DMA at all.
  - "generic": any other [B, S, S] bool mask; dense scores multiplied by the
    0/1 mask (streamed as bf16).
"""

import numpy as np
import ml_dtypes

import concourse.bass as bass
import concourse.mybir as mybir
import concourse.tile as tile
from concourse import bacc
from concourse.bass_utils import run_bass_kernel_spmd
from concourse.masks import make_identity, make_upper_triangular

B, S, D, H = 2, 2048, 1024, 16
HD = D // H
N_TRACKS = 2
BAR = 64
TL = S // N_TRACKS            # 1024 tokens per track
N_CORES = 8
HPC = H // (N_CORES // B)     # 4 heads per core
FPC = HPC * HD                # 256 features per core
DT = mybir.dt
BF16 = ml_dtypes.bfloat16

_cache: dict = {}


def _structured_mask() -> np.ndarray:
    idx = np.arange(S)
    track = idx // TL
    pos = idx % TL
    bar = pos // BAR
    same_track = track[:, None] == track[None, :]
    causal = pos[:, None] >= pos[None, :]
    same_bar = bar[:, None] == bar[None, :]
    return (same_track & causal) | (~same_track & same_bar)


def _build(structured: bool):
    nc = bacc.Bacc()
    f32, bf16 = DT.float32, DT.bfloat16

    xT = nc.declare_dram_parameter("xT", [128, 8, S], bf16, isOutput=False)
    wq = nc.declare_dram_parameter("wq", [128, 8, FPC], bf16, isOutput=False)
    wk = nc.declare_dram_parameter("wk", [128, 8, FPC], bf16, isOutput=False)
    wv = nc.declare_dram_parameter("wv", [128, 8, FPC], bf16, isOutput=False)
    wo = nc.declare_dram_parameter("wo", [128, 2, D], bf16, isOutput=False)
    bq = nc.declare_dram_parameter("bq", [128, 2], f32, isOutput=False)
    bk = nc.declare_dram_parameter("bk", [128, 2], f32, isOutput=False)
    if structured:
        bm_d = nc.declare_dram_parameter("bm", [128, 128], bf16, isOutput=False)
    else:
        maskT = nc.declare_dram_parameter("maskT", [S, S], bf16, isOutput=False)
    out_d = nc.declare_dram_parameter("out", [S, D], f32, isOutput=True)

    with tile.TileContext(nc) as tc:
        with (
            tc.tile_pool(name="consts", bufs=1) as consts,
            tc.tile_pool(name="pp", bufs=20) as ppool,
            tc.tile_pool(name="small", bufs=6) as small,
            tc.tile_pool(name="mp", bufs=4) as mpool,
            tc.tile_pool(name="ps512", bufs=2, space="PSUM") as ps512,
            tc.tile_pool(name="scps", bufs=2, space="PSUM") as scps,
        ):
            Copy = mybir.ActivationFunctionType.Copy
            Exp = mybir.ActivationFunctionType.Exp

            xt_sb = consts.tile([128, 8, S], bf16)
            nc.sync.dma_start(out=xt_sb, in_=xT[:, :, :])
            wq_sb = consts.tile([128, 8, FPC], bf16)
            nc.sync.dma_start(out=wq_sb, in_=wq[:, :, :])
            wk_sb = consts.tile([128, 8, FPC], bf16)
            nc.sync.dma_start(out=wk_sb, in_=wk[:, :, :])
            wv_sb = consts.tile([128, 8, FPC], bf16)
            nc.sync.dma_start(out=wv_sb, in_=wv[:, :, :])
            wo_sb = consts.tile([128, 2, D], bf16)
            nc.sync.dma_start(out=wo_sb, in_=wo[:, :, :])
            bq_sb = consts.tile([128, 2], f32)
            nc.sync.dma_start(out=bq_sb, in_=bq[:, :])
            bk_sb = consts.tile([128, 2], f32)
            nc.sync.dma_start(out=bk_sb, in_=bk[:, :])

            m20 = consts.tile([128, 1], f32)
            nc.vector.memset(m20, -20.0)
            ident = consts.tile([128, 128], bf16)
            make_identity(nc, ident)
            if structured:
                tri = consts.tile([128, 128], bf16)
                make_upper_triangular(nc, tri, val=1.0, diag=True)
                # cross-track 128x128 tile is block-diagonal in 64-token bars
                bm = consts.tile([128, 128], bf16)
                nc.sync.dma_start(out=bm, in_=bm_d[:, :])

            qT_sb = consts.tile([128, 2, S], bf16)
            kT_sb = consts.tile([128, 2, S], bf16)
            # v' tiles: per k-tile, 4 heads x (64 v columns + ones column)
            v_sb = consts.tile([128, 16, HPC * (HD + 1)], bf16)
            v4 = v_sb.rearrange("p k (h c) -> p k h c", c=HD + 1)
            nc.gpsimd.memset(v4[:, :, :, HD : HD + 1], 1.0)
            attn_sb = consts.tile([128, 16, FPC], bf16)
            attnT_sb = consts.tile([128, 2, S], bf16)

            # ---------------- projections ----------------
            for dst, w_sb, b_sb in ((qT_sb, wq_sb, bq_sb), (kT_sb, wk_sb, bk_sb)):
                for ft in range(2):
                    for qb in range(4):
                        ps = ps512.tile([128, 512], f32, tag="ps512")
                        for dt_i in range(8):
                            nc.tensor.matmul(
                                ps,
                                w_sb[:, dt_i, ft * 128 : (ft + 1) * 128],
                                xt_sb[:, dt_i, qb * 512 : (qb + 1) * 512],
                                start=(dt_i == 0),
                                stop=(dt_i == 7),
                            )
                        nc.vector.tensor_scalar_add(
                            out=dst[:, ft, qb * 512 : (qb + 1) * 512],
                            in0=ps,
                            scalar1=b_sb[:, ft : ft + 1],
                        )
            for tb in range(16):
                ps = ps512.tile([128, FPC], f32, tag="ps512")
                for dt_i in range(8):
                    nc.tensor.matmul(
                        ps,
                        xt_sb[:, dt_i, tb * 128 : (tb + 1) * 128],
                        wv_sb[:, dt_i, :],
                        start=(dt_i == 0),
                        stop=(dt_i == 7),
                    )
                nc.vector.tensor_copy(
                    out=v4[:, tb, :, 0:HD],
                    in_=ps.rearrange("p (h c) -> p h c", c=HD),
                )

            # ---------------- attention ----------------
            NQC = TL // 128  # 8 q-chunks per track
            for h in range(HPC):
                fth, hh = h // 2, h % 2
                prow = slice(hh * 64, hh * 64 + 64)

                def _av_block(tbg, mms, h=h, fth=fth, prow=prow):
                    av = ps512.tile([128, HD + 1], f32, tag="ps512")
                    for j, (lh, ktg) in enumerate(mms):
                        nc.tensor.matmul(
                            av, lh, v4[:, ktg, h, :],
                            start=(j == 0), stop=(j == len(mms) - 1),
                        )
                    r = small.tile([128, 1], f32, tag="recip")
                    nc.vector.reciprocal(r, av[:, HD : HD + 1])
                    nc.vector.tensor_scalar_mul(
                        attn_sb[:, tbg, h * 64 : (h + 1) * 64], av[:, 0:HD], r
                    )
                    tp = ps512.tile([128, 128], bf16, tag="ps512")
                    nc.tensor.transpose(
                        tp[0:64, :], attn_sb[:, tbg, h * 64 : (h + 1) * 64], ident
                    )
                    nc.vector.tensor_copy(
                        out=attnT_sb[prow, fth, tbg * 128 : (tbg + 1) * 128],
                        in_=tp[0:64, :],
                    )

                ptiles = {}
                if structured:
                    # per k-tile (track t, local i): q columns cover
                    # [own-track q from 128*i to track end | cross-track 128]
                    for t in range(2):
                        for i in range(NQC):
                            wA = TL - 128 * i
                            wT = wA + 128
                            ktg = t * NQC + i
                            sc = scps.tile([128, 1152], f32, tag="scps")
                            lhsT = kT_sb[prow, fth, ktg * 128 : (ktg + 1) * 128]
                            col = 0
                            while col < wA:
                                wseg = min(512, wA - col)
                                qg = t * TL + 128 * i + col
                                nc.tensor.matmul(
                                    sc[:, col : col + wseg],
                                    lhsT,
                                    qT_sb[prow, fth, qg : qg + wseg],
                                    start=True,
                                    stop=True,
                                )
                                col += wseg
                            qg = (1 - t) * TL + 128 * i
                            nc.tensor.matmul(
                                sc[:, wA:wT],
                                lhsT,
                                qT_sb[prow, fth, qg : qg + 128],
                                start=True,
                                stop=True,
                            )
                            pt = ppool.tile([128, 1152], bf16, tag="pp")
                            nc.scalar.activation(
                                out=pt[:, 0:wT], in_=sc[:, 0:wT], func=Exp,
                                bias=m20, scale=1.0,
                            )
                            nc.vector.tensor_mul(pt[:, 0:128], pt[:, 0:128], tri)
                            nc.vector.tensor_mul(pt[:, wA:wT], pt[:, wA:wT], bm)
                            ptiles[(t, i)] = pt
                    for t in range(2):
                        for qc in range(NQC):
                            mms = []
                            for i in range(qc + 1):
                                mms.append(
                                    (ptiles[(t, i)][:, 128 * (qc - i) : 128 * (qc - i) + 128],
                                     t * NQC + i)
                                )
                            wAc = TL - 128 * qc
                            mms.append(
                                (ptiles[(1 - t, qc)][:, wAc : wAc + 128],
                                 (1 - t) * NQC + qc)
                            )
                            _av_block(t * NQC + qc, mms)
                else:
                    # dense: per q-half, all k-tiles then the AV for that half
                    for half in range(2):
                        ptiles = {}
                        for ktg in range(16):
                            lhsT = kT_sb[prow, fth, ktg * 128 : (ktg + 1) * 128]
                            sc = scps.tile([128, 1152], f32, tag="scps")
                            for seg in range(2):
                                qg = half * 1024 + seg * 512
                                nc.tensor.matmul(
                                    sc[:, seg * 512 : (seg + 1) * 512],
                                    lhsT,
                                    qT_sb[prow, fth, qg : qg + 512],
                                    start=True,
                                    stop=True,
                                )
                            pt = ppool.tile([128, 1152], bf16, tag="pp")
                            nc.scalar.activation(
                                out=pt[:, 0:1024], in_=sc[:, 0:1024], func=Exp,
                                bias=m20, scale=1.0,
                            )
                            mt = mpool.tile([128, 1024], bf16, tag="mp")
                            nc.sync.dma_start(
                                out=mt,
                                in_=maskT[ktg * 128 : (ktg + 1) * 128,
                                          half * 1024 : (half + 1) * 1024],
                            )
                            nc.vector.tensor_mul(pt[:, 0:1024], pt[:, 0:1024], mt)
                            ptiles[ktg] = pt
                        for qc in range(NQC):
                            mms = [
                                (ptiles[ktg][:, 128 * qc : 128 * qc + 128], ktg)
                                for ktg in range(16)
                            ]
                            _av_block(half * NQC + qc, mms)

            # ---------------- output projection ----------------
            for tb in range(16):
                for ob in range(2):
                    ps = ps512.tile([128, 512], f32, tag="ps512")
                    for ftt in range(2):
                        nc.tensor.matmul(
                            ps,
                            attnT_sb[:, ftt, tb * 128 : (tb + 1) * 128],
                            wo_sb[:, ftt, ob * 512 : (ob + 1) * 512],
                            start=(ftt == 0),
                            stop=(ftt == 1),
                        )
                    ot = small.tile([128, 512], f32, tag="outstage")
                    nc.vector.tensor_copy(out=ot, in_=ps)
                    nc.sync.dma_start(
                        out=out_d[tb * 128 : (tb + 1) * 128, ob * 512 : (ob + 1) * 512],
                        in_=ot,
                    )
    nc.finalize()
    return nc


def _get_nc(structured: bool):
    key = "structured" if structured else "generic"
    if key not in _cache:
        _cache[key] = _build(structured)
    return _cache[key]


def kernel(x, cross_track_mask, w_qkv, b_qkv, w_out, b_out):
    x = np.asarray(x, dtype=np.float32)
    mask = np.asarray(cross_track_mask).astype(bool)
    w_qkv = np.asarray(w_qkv, dtype=np.float32)
    b_qkv = np.asarray(b_qkv, dtype=np.float32)
    w_out = np.asarray(w_out, dtype=np.float32)
    b_out = np.asarray(b_out, dtype=np.float32)

    structured = bool(np.array_equal(mask, np.broadcast_to(_structured_mask(), mask.shape)))
    nc = _get_nc(structured)

    scale = 1.0 / np.sqrt(np.float32(HD))
    b_v = b_qkv[2 * D :]
    b_out_adj = (b_out + b_v @ w_out).astype(np.float32)

    in_maps = []
    for c in range(N_CORES):
        b = c // (N_CORES // B)
        g = c % (N_CORES // B)
        fs = slice(g * FPC, (g + 1) * FPC)

        xT_c = np.ascontiguousarray(
            x[b].T.reshape(8, 128, S).transpose(1, 0, 2)
        ).astype(BF16)

        def wslice(off):
            w = w_qkv[:, off + g * FPC : off + (g + 1) * FPC]
            return np.ascontiguousarray(
                w.reshape(8, 128, FPC).transpose(1, 0, 2)
            )

        wq_c = (wslice(0) * scale).astype(BF16)
        wk_c = wslice(D).astype(BF16)
        wv_c = wslice(2 * D).astype(BF16)
        bq_c = np.ascontiguousarray(
            (b_qkv[fs] * scale).reshape(2, 128).T
        ).astype(np.float32)
        bk_c = np.ascontiguousarray(
            b_qkv[D + g * FPC : D + (g + 1) * FPC].reshape(2, 128).T
        ).astype(np.float32)
        wo_c = np.ascontiguousarray(
            w_out[fs].reshape(2, 128, D).transpose(1, 0, 2)
        ).astype(BF16)

        m = {
            "xT": xT_c,
            "wq": wq_c,
            "wk": wk_c,
            "wv": wv_c,
            "wo": wo_c,
            "bq": bq_c,
            "bk": bk_c,
        }
        if structured:
            ar = np.arange(128)
            m["bm"] = ((ar[:, None] // BAR) == (ar[None, :] // BAR)).astype(BF16)
        else:
            m["maskT"] = np.ascontiguousarray(mask[b].T).astype(BF16)
        in_maps.append(m)

    res = run_bass_kernel_spmd(nc, in_maps, list(range(N_CORES)))

    out = np.empty((B, S, D), dtype=np.float32)
    gpb = N_CORES // B
    for b in range(B):
        acc = res.results[b * gpb]["out"].astype(np.float32)
        for g in range(1, gpb):
            acc = acc + res.results[b * gpb + g]["out"]
        out[b] = acc + b_out_adj
    return out


# revision 20
# speedup vs baseline: 1.0599x; 1.0599x over previous
"""CrossTrackAttention Trainium2 kernel (8-core SPMD, batch x head-group sharding).

Reference computation (B=2, S=2048, D=1024, H=16, HD=64):
    qkv = x @ w_qkv + b_qkv
    q, k, v per head; scores = q k^T / sqrt(HD); masked softmax with a
    [B, S, S] bool mask; out = (attn @ v) @ w_out + b_out.

Sharding: core c handles batch c//4 and heads [4*(c%4), 4*(c%4)+4).  The
[B,H,S,S] score tensor partitions cleanly along B and H, so there are no
cross-device comms; the per-core partial outputs (each over 4 heads' feature
rows of w_out) are summed on the host.

Device algorithm per core (transpose-free flash attention):
  - host passes x^T, so QKV projections produce q^T/k^T in [feature, token]
    layout directly (lhsT = w block, rhs = x^T block) and v in natural
    [token, feature] layout (lhsT = x^T block, rhs = w block).
  - scores are computed transposed, s^T[k, q] (lhsT = k^T slice, rhs = q^T
    slice), so softmax numerator exp(s - 20) runs on ACT over wide-q tiles
    and the attention@V matmul consumes p^T tiles as lhsT with no transposes.
  - a ones column appended to V accumulates the softmax denominator in the
    same PSUM accumulation group; out = p~ @ [v|1] then row-scaled by the
    reciprocal of the denominator (softmax is shift invariant, so the fixed
    -20 shift cancels).
  - 1/sqrt(HD) is folded into w_q/b_q on the host; b_v is folded into the
    output bias (sum_k softmax = 1  =>  +b_v passes through attention), so
    the device never touches b_v.

Two compiled variants:
  - "structured": the cross-track mask of the reference's setup_inputs()
    (causal within each of 2 tracks of 1024 tokens + bidirectional same-bar
    cross-track attention, BAR=64).  Block-sparse schedule with a constant
    128x128 triangular tile for the causal diagonal; no mask DMA at all.
  - "generic": any other [B, S, S] bool mask; dense scores multiplied by the
    0/1 mask (streamed as bf16).
"""

import numpy as np
import ml_dtypes

import concourse.bass as bass
import concourse.mybir as mybir
import concourse.tile as tile
from concourse import bacc
from concourse.bass_utils import run_bass_kernel_spmd
from concourse.masks import make_identity, make_upper_triangular

B, S, D, H = 2, 2048, 1024, 16
HD = D // H
N_TRACKS = 2
BAR = 64
TL = S // N_TRACKS            # 1024 tokens per track
N_CORES = 8
HPC = H // (N_CORES // B)     # 4 heads per core
FPC = HPC * HD                # 256 features per core
DT = mybir.dt
BF16 = ml_dtypes.bfloat16

_cache: dict = {}


def _structured_mask() -> np.ndarray:
    idx = np.arange(S)
    track = idx // TL
    pos = idx % TL
    bar = pos // BAR
    same_track = track[:, None] == track[None, :]
    causal = pos[:, None] >= pos[None, :]
    same_bar = bar[:, None] == bar[None, :]
    return (same_track & causal) | (~same_track & same_bar)


def _build(structured: bool):
    nc = bacc.Bacc()
    f32, bf16 = DT.float32, DT.bfloat16

    xT = nc.declare_dram_parameter("xT", [128, 8, S], bf16, isOutput=False)
    wq = nc.declare_dram_parameter("wq", [128, 8, FPC], bf16, isOutput=False)
    wk = nc.declare_dram_parameter("wk", [128, 8, FPC], bf16, isOutput=False)
    wv = nc.declare_dram_parameter("wv", [128, 8, FPC], bf16, isOutput=False)
    wo = nc.declare_dram_parameter("wo", [128, 2, D], bf16, isOutput=False)
    bq = nc.declare_dram_parameter("bq", [128, 2], f32, isOutput=False)
    bk = nc.declare_dram_parameter("bk", [128, 2], f32, isOutput=False)
    if structured:
        bm_d = nc.declare_dram_parameter("bm", [128, 128], bf16, isOutput=False)
    else:
        maskT = nc.declare_dram_parameter("maskT", [S, S], bf16, isOutput=False)
    out_d = nc.declare_dram_parameter("out", [S, D], f32, isOutput=True)

    with tile.TileContext(nc) as tc:
        with (
            tc.tile_pool(name="consts", bufs=1) as consts,
            tc.tile_pool(name="pp", bufs=36) as ppool,
            tc.tile_pool(name="small", bufs=6) as small,
            tc.tile_pool(name="mp", bufs=4) as mpool,
            tc.tile_pool(name="ps512", bufs=2, space="PSUM") as ps512,
            tc.tile_pool(name="scps", bufs=2, space="PSUM") as scps,
            tc.tile_pool(name="avps", bufs=2, space="PSUM") as avps,
        ):
            Copy = mybir.ActivationFunctionType.Copy
            Exp = mybir.ActivationFunctionType.Exp

            xt_sb = consts.tile([128, 8, S], bf16)
            nc.sync.dma_start(out=xt_sb, in_=xT[:, :, :])
            wq_sb = consts.tile([128, 8, FPC], bf16)
            nc.sync.dma_start(out=wq_sb, in_=wq[:, :, :])
            wk_sb = consts.tile([128, 8, FPC], bf16)
            nc.sync.dma_start(out=wk_sb, in_=wk[:, :, :])
            wv_sb = consts.tile([128, 8, FPC], bf16)
            nc.sync.dma_start(out=wv_sb, in_=wv[:, :, :])
            wo_sb = consts.tile([128, 2, D], bf16)
            nc.sync.dma_start(out=wo_sb, in_=wo[:, :, :])
            bq_sb = consts.tile([128, 2], f32)
            nc.sync.dma_start(out=bq_sb, in_=bq[:, :])
            bk_sb = consts.tile([128, 2], f32)
            nc.sync.dma_start(out=bk_sb, in_=bk[:, :])

            m20 = consts.tile([128, 1], f32)
            nc.vector.memset(m20, -20.0)
            ident = consts.tile([128, 128], bf16)
            make_identity(nc, ident)
            if structured:
                tri = consts.tile([128, 128], bf16)
                make_upper_triangular(nc, tri, val=1.0, diag=True)
                # cross-track 128x128 tile is block-diagonal in 64-token bars
                bm = consts.tile([128, 128], bf16)
                nc.sync.dma_start(out=bm, in_=bm_d[:, :])

            qT_sb = consts.tile([128, 2, S], bf16)
            kT_sb = consts.tile([128, 2, S], bf16)
            # v' tiles: per k-tile, 4 heads x (64 v columns + ones column)
            v_sb = consts.tile([128, 16, HPC * (HD + 1)], bf16)
            v4 = v_sb.rearrange("p k (h c) -> p k h c", c=HD + 1)
            nc.gpsimd.memset(v4[:, :, :, HD : HD + 1], 1.0)
            attn_sb = consts.tile([128, 16, FPC], bf16)
            attnT_sb = consts.tile([128, 2, S], bf16)

            # ---------------- projections ----------------
            for dst, w_sb, b_sb in ((qT_sb, wq_sb, bq_sb), (kT_sb, wk_sb, bk_sb)):
                for ft in range(2):
                    for qb in range(4):
                        ps = ps512.tile([128, 512], f32, tag="ps512")
                        for dt_i in range(8):
                            nc.tensor.matmul(
                                ps,
                                w_sb[:, dt_i, ft * 128 : (ft + 1) * 128],
                                xt_sb[:, dt_i, qb * 512 : (qb + 1) * 512],
                                start=(dt_i == 0),
                                stop=(dt_i == 7),
                            )
                        nc.vector.tensor_scalar_add(
                            out=dst[:, ft, qb * 512 : (qb + 1) * 512],
                            in0=ps,
                            scalar1=b_sb[:, ft : ft + 1],
                        )
            for tb in range(16):
                ps = ps512.tile([128, FPC], f32, tag="ps512")
                for dt_i in range(8):
                    nc.tensor.matmul(
                        ps,
                        xt_sb[:, dt_i, tb * 128 : (tb + 1) * 128],
                        wv_sb[:, dt_i, :],
                        start=(dt_i == 0),
                        stop=(dt_i == 7),
                    )
                nc.any.tensor_copy(
                    out=v4[:, tb, :, 0:HD],
                    in_=ps.rearrange("p (h c) -> p h c", c=HD),
                )

            # ---------------- attention ----------------
            NQC = TL // 128  # 8 q-chunks per track
            for h in range(HPC):
                fth, hh = h // 2, h % 2
                prow = slice(hh * 64, hh * 64 + 64)

                def _av_block(tbg, mms, h=h, fth=fth, prow=prow):
                    av = avps.tile([128, HD + 1], f32, tag="av")
                    for j, (lh, ktg) in enumerate(mms):
                        nc.tensor.matmul(
                            av, lh, v4[:, ktg, h, :],
                            start=(j == 0), stop=(j == len(mms) - 1),
                        )
                    r = small.tile([128, 1], f32, tag="recip")
                    nc.vector.reciprocal(r, av[:, HD : HD + 1])
                    nc.vector.tensor_scalar_mul(
                        attn_sb[:, tbg, h * 64 : (h + 1) * 64], av[:, 0:HD], r
                    )
                    tp = avps.tile([128, 128], bf16, tag="av")
                    nc.tensor.transpose(
                        tp[0:64, :], attn_sb[:, tbg, h * 64 : (h + 1) * 64], ident
                    )
                    nc.vector.tensor_copy(
                        out=attnT_sb[prow, fth, tbg * 128 : (tbg + 1) * 128],
                        in_=tp[0:64, :],
                    )

                ptiles = {}
                if structured:
                    # per k-tile (track t, local i): q columns cover
                    # [own-track q from 128*i to track end | cross-track 128]
                    pcross = {}
                    for t in range(2):
                        for i in range(NQC):
                            wA = TL - 128 * i
                            wT = wA + 128
                            ktg = t * NQC + i
                            lhsT = kT_sb[prow, fth, ktg * 128 : (ktg + 1) * 128]
                            split = wT > 1024
                            scw = wA if split else wT
                            sc = scps.tile([128, 1024], f32, tag="scps")
                            col = 0
                            while col < wA:
                                wseg = min(512, wA - col)
                                qg = t * TL + 128 * i + col
                                nc.tensor.matmul(
                                    sc[:, col : col + wseg],
                                    lhsT,
                                    qT_sb[prow, fth, qg : qg + wseg],
                                    start=True,
                                    stop=True,
                                )
                                col += wseg
                            qg = (1 - t) * TL + 128 * i
                            if split:
                                scx = avps.tile([128, 128], f32, tag="av")
                                nc.tensor.matmul(
                                    scx, lhsT,
                                    qT_sb[prow, fth, qg : qg + 128],
                                    start=True, stop=True,
                                )
                                px = small.tile([128, 128], bf16, tag="ppx")
                                nc.scalar.activation(
                                    out=px, in_=scx, func=Exp, bias=m20, scale=1.0,
                                )
                                nc.vector.tensor_mul(px, px, bm)
                                pcross[(t, i)] = (px, 0)
                            else:
                                nc.tensor.matmul(
                                    sc[:, wA:wT], lhsT,
                                    qT_sb[prow, fth, qg : qg + 128],
                                    start=True, stop=True,
                                )
                            pt = ppool.tile([128, 1024], bf16, tag="pp")
                            nc.scalar.activation(
                                out=pt[:, 0:scw], in_=sc[:, 0:scw], func=Exp,
                                bias=m20, scale=1.0,
                            )
                            nc.vector.tensor_mul(pt[:, 0:128], pt[:, 0:128], tri)
                            if not split:
                                nc.vector.tensor_mul(pt[:, wA:wT], pt[:, wA:wT], bm)
                                pcross[(t, i)] = (pt, wA)
                            ptiles[(t, i)] = pt
                    for t in range(2):
                        for qc in range(NQC):
                            mms = []
                            for i in range(qc + 1):
                                mms.append(
                                    (ptiles[(t, i)][:, 128 * (qc - i) : 128 * (qc - i) + 128],
                                     t * NQC + i)
                                )
                            pxt, xoff = pcross[(1 - t, qc)]
                            mms.append(
                                (pxt[:, xoff : xoff + 128], (1 - t) * NQC + qc)
                            )
                            _av_block(t * NQC + qc, mms)
                else:
                    # dense: per q-half, all k-tiles then the AV for that half
                    for half in range(2):
                        ptiles = {}
                        for ktg in range(16):
                            lhsT = kT_sb[prow, fth, ktg * 128 : (ktg + 1) * 128]
                            sc = scps.tile([128, 1024], f32, tag="scps")
                            for seg in range(2):
                                qg = half * 1024 + seg * 512
                                nc.tensor.matmul(
                                    sc[:, seg * 512 : (seg + 1) * 512],
                                    lhsT,
                                    qT_sb[prow, fth, qg : qg + 512],
                                    start=True,
                                    stop=True,
                                )
                            pt = ppool.tile([128, 1024], bf16, tag="pp")
                            nc.scalar.activation(
                                out=pt[:, 0:1024], in_=sc[:, 0:1024], func=Exp,
                                bias=m20, scale=1.0,
                            )
                            mt = mpool.tile([128, 1024], bf16, tag="mp")
                            nc.sync.dma_start(
                                out=mt,
                                in_=maskT[ktg * 128 : (ktg + 1) * 128,
                                          half * 1024 : (half + 1) * 1024],
                            )
                            nc.vector.tensor_mul(pt[:, 0:1024], pt[:, 0:1024], mt)
                            ptiles[ktg] = pt
                        for qc in range(NQC):
                            mms = [
                                (ptiles[ktg][:, 128 * qc : 128 * qc + 128], ktg)
                                for ktg in range(16)
                            ]
                            _av_block(half * NQC + qc, mms)

            # ---------------- output projection ----------------
            for tb in range(16):
                for ob in range(2):
                    ps = ps512.tile([128, 512], f32, tag="ps512")
                    for ftt in range(2):
                        nc.tensor.matmul(
                            ps,
                            attnT_sb[:, ftt, tb * 128 : (tb + 1) * 128],
                            wo_sb[:, ftt, ob * 512 : (ob + 1) * 512],
                            start=(ftt == 0),
                            stop=(ftt == 1),
                        )
                    ot = small.tile([128, 512], f32, tag="outstage")
                    nc.any.tensor_copy(out=ot, in_=ps)
                    nc.sync.dma_start(
                        out=out_d[tb * 128 : (tb + 1) * 128, ob * 512 : (ob + 1) * 512],
                        in_=ot,
                    )
    nc.finalize()
    return nc


def _get_nc(structured: bool):
    key = "structured" if structured else "generic"
    if key not in _cache:
        _cache[key] = _build(structured)
    return _cache[key]


def kernel(x, cross_track_mask, w_qkv, b_qkv, w_out, b_out):
    x = np.asarray(x, dtype=np.float32)
    mask = np.asarray(cross_track_mask).astype(bool)
    w_qkv = np.asarray(w_qkv, dtype=np.float32)
    b_qkv = np.asarray(b_qkv, dtype=np.float32)
    w_out = np.asarray(w_out, dtype=np.float32)
    b_out = np.asarray(b_out, dtype=np.float32)

    structured = bool(np.array_equal(mask, np.broadcast_to(_structured_mask(), mask.shape)))
    nc = _get_nc(structured)

    scale = 1.0 / np.sqrt(np.float32(HD))
    b_v = b_qkv[2 * D :]
    b_out_adj = (b_out + b_v @ w_out).astype(np.float32)

    in_maps = []
    for c in range(N_CORES):
        b = c // (N_CORES // B)
        g = c % (N_CORES // B)
        fs = slice(g * FPC, (g + 1) * FPC)

        xT_c = np.ascontiguousarray(
            x[b].T.reshape(8, 128, S).transpose(1, 0, 2)
        ).astype(BF16)

        def wslice(off):
            w = w_qkv[:, off + g * FPC : off + (g + 1) * FPC]
            return np.ascontiguousarray(
                w.reshape(8, 128, FPC).transpose(1, 0, 2)
            )

        wq_c = (wslice(0) * scale).astype(BF16)
        wk_c = wslice(D).astype(BF16)
        wv_c = wslice(2 * D).astype(BF16)
        bq_c = np.ascontiguousarray(
            (b_qkv[fs] * scale).reshape(2, 128).T
        ).astype(np.float32)
        bk_c = np.ascontiguousarray(
            b_qkv[D + g * FPC : D + (g + 1) * FPC].reshape(2, 128).T
        ).astype(np.float32)
        wo_c = np.ascontiguousarray(
            w_out[fs].reshape(2, 128, D).transpose(1, 0, 2)
        ).astype(BF16)

        m = {
            "xT": xT_c,
            "wq": wq_c,
            "wk": wk_c,
            "wv": wv_c,
            "wo": wo_c,
            "bq": bq_c,
            "bk": bk_c,
        }
        if structured:
            ar = np.arange(128)
            m["bm"] = ((ar[:, None] // BAR) == (ar[None, :] // BAR)).astype(BF16)
        else:
            m["maskT"] = np.ascontiguousarray(mask[b].T).astype(BF16)
        in_maps.append(m)

    res = run_bass_kernel_spmd(nc, in_maps, list(range(N_CORES)))

    out = np.empty((B, S, D), dtype=np.float32)
    gpb = N_CORES // B
    for b in range(B):
        acc = res.results[b * gpb]["out"].astype(np.float32)
        for g in range(1, gpb):
            acc = acc + res.results[b * gpb + g]["out"]
        out[b] = acc + b_out_adj
    return out
